# revision 2
# baseline (speedup 1.0000x reference)
"""Trainium2 Bass kernel for Llama-style GQA attention (B=2,S=2048,H=4096,NH=32,NKV=8,HD=128).

v2 sharding: tensor-parallel over heads for QKV+attention (core c owns Q-heads
4c..4c+3 and GQA KV-head c), then a per-slice bf16 AllGather of the normalized
attention outputs and a column-parallel O-projection (core c owns output
columns 512c..512c+511 with the full-depth Wo shard resident in SBUF).
kernel(**inputs) takes full inputs, returns the full output.
"""

import math
import os
from contextlib import ExitStack

import numpy as np

B, S, H = 2, 2048, 4096
NH, NKV, HD = 32, 8, 128
THETA = 1000000.0
NCORES = 8
QH = NH // NCORES            # 4 q-heads per core
TOK = B * S                  # 4096 tokens (flattened batch*seq)
QO = QH * HD                 # 512 q-out dims per core
FO = H // NCORES             # 512 output columns per core (O-proj shard)
TT = TOK // 128              # 32 token tiles of 128
TS = TOK // 512              # 8 token slices of 512
SB = S // 512                # 4 q-slices of 512 per batch
KTB = S // 128               # 16 k-tiles of 128 per batch
HT = H // 128                # 32 hidden tiles

LAST_EXEC_NS = None
LAST_RESULT = None

_compiled = {}


def _build():
    import concourse.bass as bass
    import concourse.mybir as mybir
    import concourse.tile as tile
    from concourse import bacc

    f32 = mybir.dt.float32
    f32r = mybir.dt.float32r            # fp32 w/ 11-bit mantissa: 1 PE cyc/row
    bf = mybir.dt.bfloat16
    nc = bacc.Bacc("TRN2", target_bir_lowering=False, debug=False,
                   num_devices=NCORES)

    def inp(name, shape, dt=f32):
        return nc.dram_tensor(name, shape, dt, kind="ExternalInput").ap()

    # hidden transposed and host-pre-tiled: xTt[ti] is a contiguous
    # (H, 512) block for token slice ti -> single-burst DMA tiles
    xTt = inp("xTt", (TS, H, 512), bf)
    # weight shards host-permuted to SBUF-resident layout [p, tile, out]
    wqP = inp("wqP", (128, HT, QO), bf)
    wkP = inp("wkP", (128, HT, HD), bf)
    wvP = inp("wvP", (128, HT, HD), bf)
    woP = inp("woP", (128, HT, FO), bf)   # full-depth Wo, this core's 512 cols
    bqP = inp("bqP", (128, QH))         # bq shard as [d, head]
    bkP = inp("bkP", (128, 1))
    bvP = inp("bvP", (128, 1))
    boP = inp("boP", (1, FO))           # this core's bo slice
    cosT = inp("cosT", (HD, TOK), bf)
    sinT = inp("sinT", (HD, TOK), bf)
    rotM = inp("rotM", (HD, HD), bf)  # lhsT for rotate_half_interleaved
    ident = inp("ident", (128, 128), f32r)
    ones = inp("ones", (128, 128), bf)
    onesf = inp("onesf", (128, 128))
    maskI = inp("maskI", (128, 128), bf)  # lower-triangle keep mask

    out = nc.dram_tensor("out", (TOK, FO), f32, kind="ExternalOutput").ap()
    # per-chunk attention-output exchange: my 512 head-dims in; all cores'
    # 4096 head-dims out (rank-major rows). Interior slices use 2 chunks of
    # 256 tokens; the edge slices (0 and 7) use 4 chunks of 128 tokens so the
    # pipeline-fill bubble and the drain tail are shorter.
    NCH = [4 if t == 0 else 2 for t in range(TS)]
    att_ins, ag_outs = [], []
    for t in range(TS):
        csz = 512 // NCH[t]
        att_ins.append([nc.dram_tensor(f"att_in{t}_{c}", (QO, csz), bf,
                                       kind="Internal").ap()
                        for c in range(NCH[t])])
        ag_outs.append([nc.dram_tensor(f"ag_out{t}_{c}", (NCORES * QO, csz),
                                       bf, kind="Internal",
                                       addr_space="Shared").ap()
                        for c in range(NCH[t])])

    inv_sqrt_hd = 1.0 / math.sqrt(HD)

    def mm(out, lhsT, rhs, **kw):
        nc.tensor.matmul(out, lhsT, rhs, **kw)

    with tile.TileContext(nc) as tc, ExitStack() as stk:
        # ---------------- constants + persistent activations ----------------
        cpool = stk.enter_context(tc.tile_pool(name="consts", bufs=1))
        apool = stk.enter_context(tc.tile_pool(name="acts", bufs=1))

        # resident QKV weight shards first (needed earliest): [p, h tile, out].
        # Split fine-grained so the first QKV matmuls start after ~2MB of DMA;
        # remaining parts are emitted inside the ti==0 h-loop, interleaved
        # with the xt tile stream, each ahead of the matmul that needs it.
        bq_sb = cpool.tile([128, QH], f32)
        nc.sync.dma_start(bq_sb[:], bqP[:])
        bk_sb = cpool.tile([128, 1], f32)
        nc.sync.dma_start(bk_sb[:], bkP[:])
        bv_sb = cpool.tile([128, 1], f32)
        nc.sync.dma_start(bv_sb[:], bvP[:])
        WQP, WKP = HT // 4, HT // 2      # h-tiles per wq / wk / wv part
        wq_res = [apool.tile([128, WQP, QO], bf, name=f"wq_res{i}")
                  for i in range(4)]
        wk_res = [apool.tile([128, WKP, HD], bf, name=f"wk_res{i}")
                  for i in range(2)]
        wv_res = [apool.tile([128, WKP, HD], bf, name=f"wv_res{i}")
                  for i in range(2)]
        nc.sync.dma_start(wq_res[0][:], wqP[:, 0:WQP, :])
        nc.sync.dma_start(wk_res[0][:], wkP[:, 0:WKP, :])
        nc.sync.dma_start(wv_res[0][:], wvP[:, 0:WKP, :])
        # later-needed constants go on the scalar DMA queue so they don't
        # delay the sync queue's xt tile stream for slice 0; cos/sin and the
        # second wq half are deferred into the ti==0 body for the same reason
        rot_sb = cpool.tile([128, 128], bf)
        nc.scalar.dma_start(rot_sb[:], rotM[:])
        id_sb = cpool.tile([128, 128], f32r)
        nc.scalar.dma_start(id_sb[:], ident[:])
        ones_sb = cpool.tile([128, 128], bf)
        nc.scalar.dma_start(ones_sb[:], ones[:])
        onesf_sb = cpool.tile([128, 128], f32)
        nc.scalar.dma_start(onesf_sb[:], onesf[:])
        mask_sb = cpool.tile([128, 128], bf)
        nc.scalar.dma_start(mask_sb[:], maskI[:])
        cos_sb = cpool.tile([128, TOK], bf)
        sin_sb = cpool.tile([128, TOK], bf)
        # O-proj weights loaded inside the loop (first needed ~150us in) so
        # they don't steal HBM bandwidth from the slice-0 ramp
        wo_res = apool.tile([128, HT, FO], bf)
        bo_sb = cpool.tile([1, FO], f32)
        bo_bc = cpool.tile([128, FO], f32)

        KT = apool.tile([128, TOK], bf)        # K^T (rope'd), grows causally
        Vsb = apool.tile([128, TT, 128], bf)   # V in [t mod 128, t tile, d]

        sp = stk.enter_context(tc.tile_pool(name="streams", bufs=16))
        tp = stk.enter_context(tc.tile_pool(name="tmps", bufs=2))
        qtp = stk.enter_context(tc.tile_pool(name="qts", bufs=2))
        vtp = stk.enter_context(tc.tile_pool(name="vts", bufs=2))
        atp = stk.enter_context(tc.tile_pool(name="attw", bufs=6))
        accp = stk.enter_context(tc.tile_pool(name="accs", bufs=4))
        drp = stk.enter_context(tc.tile_pool(name="drs", bufs=2))
        anp = stk.enter_context(tc.tile_pool(name="atn", bufs=2))
        agp = stk.enter_context(tc.tile_pool(name="ags", bufs=1))
        stp = stk.enter_context(tc.tile_pool(name="ostage", bufs=2))
        pp = stk.enter_context(tc.tile_pool(name="ps", bufs=8, space="PSUM"))

        def ps_tile(shape=(128, 512), dt=f32):
            return pp.tile(list(shape), dt, name="ps", tag="ps")

        def oproj_chunk(ti, c):
            """O-projection for one token chunk of slice ti (FO cols)."""
            csz = 512 // NCH[ti]
            ag_sb = agp.tile([128, HT, 256], bf, name="ag_sb")
            nc.sync.dma_start(
                ag_sb[:, :, 0:csz],
                ag_outs[ti][c].rearrange("(dt p) t -> p dt t", p=128))
            for tt in range(csz // 128):
                op_ps = ps_tile()
                for dt in range(HT):
                    mm(op_ps[:], ag_sb[:, dt, tt * 128:(tt + 1) * 128],
                       wo_res[:, dt, :], start=(dt == 0), stop=(dt == HT - 1))
                st_t = stp.tile([128, FO], f32, name="st_t")
                nc.vector.tensor_add(st_t[:], op_ps[:], bo_bc[:])
                t0o = ti * 512 + c * csz + tt * 128
                nc.scalar.dma_start(out[t0o: t0o + 128, :], st_t[:])

        for ti in range(TS):
            b, j = ti // SB, ti % SB
            t0 = ti * 512
            # ---- QKV projection for this token slice (accumulate over h) ----
            psq = [ps_tile() for _ in range(QH)]
            psk = ps_tile()
            psv = ps_tile()
            for hi in range(HT):
                if ti == 0:
                    if hi == WQP:
                        nc.sync.dma_start(wq_res[1][:], wqP[:, WQP:2 * WQP, :])
                        nc.sync.dma_start(wk_res[1][:], wkP[:, WKP:, :])
                        nc.sync.dma_start(wv_res[1][:], wvP[:, WKP:, :])
                    elif hi == 2 * WQP:
                        nc.sync.dma_start(wq_res[2][:],
                                          wqP[:, 2 * WQP:3 * WQP, :])
                    elif hi == 3 * WQP:
                        nc.sync.dma_start(wq_res[3][:], wqP[:, 3 * WQP:, :])
                h0 = hi * 128
                xt = sp.tile([128, 512], bf, name="xt")
                nc.sync.dma_start(xt[:], xTt[ti, h0:h0 + 128, :])
                st = (hi == 0)
                en = (hi == HT - 1)
                for q in range(QH):
                    mm(psq[q][:],
                       wq_res[hi // WQP][:, hi % WQP, q * 128:(q + 1) * 128],
                       xt[:], start=st, stop=en)
                mm(psk[:], wk_res[hi // WKP][:, hi % WKP, :],
                   xt[:], start=st, stop=en)
                mm(psv[:], wv_res[hi // WKP][:, hi % WKP, :],
                   xt[:], start=st, stop=en)
            if ti == 0:
                nc.scalar.dma_start(cos_sb[:], cosT[:])
                nc.scalar.dma_start(sin_sb[:], sinT[:])

            # bias add (per-partition) while draining PSUM
            QTs = qtp.tile([128, QH, 512], bf, name="QTs")
            VTs = vtp.tile([128, 512], f32r, name="VTs")
            for q in range(QH):
                nc.scalar.add(QTs[:, q, :], psq[q][:], bq_sb[:, q:q + 1])
            nc.scalar.add(KT[:, t0:t0 + 512], psk[:], bk_sb[:, 0:1])
            nc.scalar.add(VTs[:], psv[:], bv_sb[:, 0:1])

            # rope in place on QT / KT slices
            def rope(ap_slice):
                rps = ps_tile()
                mm(rps[:], rot_sb[:], ap_slice, start=True, stop=True)
                t1 = tp.tile([128, 512], f32, name="t1")
                nc.vector.tensor_mul(t1[:], ap_slice, cos_sb[:, t0:t0 + 512])
                t2 = tp.tile([128, 512], f32, name="t2")
                nc.vector.tensor_mul(t2[:], rps[:], sin_sb[:, t0:t0 + 512])
                nc.vector.tensor_add(ap_slice, t1[:], t2[:])

            rope(KT[:, t0:t0 + 512])   # first: scores need K before all Q heads
            for q in range(QH):
                rope(QTs[:, q, :])

            # V^T -> V (PE transpose of 128x128 blocks)
            for s4 in range(4):
                g = ti * 4 + s4
                vps = pp.tile([128, 128], f32r, name="vps", tag="ps")
                nc.tensor.transpose(vps[:], VTs[:, s4 * 128:(s4 + 1) * 128],
                                    id_sb[:])
                nc.scalar.copy(Vsb[:, g, :], vps[:])

            # first half of the previous slice's O-projection chunks:
            # independent PE work that fills the rope/V-transpose window
            if ti > 0:
                for c in range(NCH[ti - 1] // 2):
                    oproj_chunk(ti - 1, c)
            else:
                # O-proj weights now that the slice-0 ramp is past its peak
                nc.scalar.dma_start(wo_res[:], woP[:])
                nc.scalar.dma_start(bo_sb[:], boP[:])
                nc.gpsimd.partition_broadcast(bo_bc[:], bo_sb[:])

            # ---- causal attention: ki-outer, head-inner (PE stays dense) ----
            nk = 4 * j + 4                # k tiles of 128 within batch b
            at_ps = [ps_tile() for _ in range(QH)]
            accs = [accp.tile([128, 512], bf, name="acc") for _ in range(QH)]
            for ki in range(nk):
                kg = b * KTB + ki
                # causal: diagonal k-tile ki only scores queries q >= s0
                s0 = (ki - 4 * j) * 128 if ki >= 4 * j else 0
                a_sbs = []
                for h in range(QH):
                    sc_ps = ps_tile()
                    mm(sc_ps[:, s0:], KT[:, kg * 128:(kg + 1) * 128],
                       QTs[:, h, s0:], start=True, stop=True)
                    a_sb = atp.tile([128, 512], bf, name="a_sb")
                    nc.scalar.activation(a_sb[:, s0:], sc_ps[:, s0:],
                                         mybir.ActivationFunctionType.Exp,
                                         scale=inv_sqrt_hd)
                    if ki >= 4 * j:
                        # triangular 128x128 block on the diagonal
                        nc.vector.tensor_mul(a_sb[:, s0:s0 + 128],
                                             a_sb[:, s0:s0 + 128], mask_sb[:])
                    a_sbs.append(a_sb)
                for h in range(QH):
                    mm(at_ps[h][:, s0:], Vsb[:, kg, :], a_sbs[h][:, s0:],
                       start=(ki == 0), stop=(ki == nk - 1))
                    # denominator partials accumulate on DVE (off PE)
                    if ki == 0:
                        nc.vector.tensor_copy(accs[h][:], a_sbs[h][:])
                    else:
                        nc.vector.tensor_add(accs[h][:, s0:],
                                             accs[h][:, s0:],
                                             a_sbs[h][:, s0:])

            # normalize: dn = colsum(acc) via 1 matmul; reciprocal; broadcast
            ATn = anp.tile([128, QH, 512], bf, name="ATn")
            for h in range(QH):
                dn_ps = ps_tile((1, 512))
                mm(dn_ps[:], ones_sb[:, 0:1], accs[h][:], start=True,
                   stop=True)
                dr = drp.tile([1, 512], f32, name="dr")
                nc.vector.reciprocal_approx_fast(dr[:], dn_ps[:])
                # broadcast 1/dn across partitions via a K=1 matmul, stage to
                # SBUF on ScalarE (keeps gpsimd free for the collectives)
                rb_ps = ps_tile()
                mm(rb_ps[:], onesf_sb[0:1, :], dr[:], start=True, stop=True)
                rb = drp.tile([128, 512], f32, name="rb")
                nc.scalar.copy(rb[:], rb_ps[:])
                nc.vector.tensor_mul(ATn[:, h, :], at_ps[h][:], rb[:])

            # ship normalized attention in token chunks; AllGather each
            csz = 512 // NCH[ti]
            for c in range(NCH[ti]):
                nc.scalar.dma_start(
                    att_ins[ti][c].rearrange("(h p) t -> p h t", p=128)[:],
                    ATn[:, :, c * csz:(c + 1) * csz])
                nc.gpsimd.collective_compute(
                    "AllGather", mybir.AluOpType.bypass,
                    replica_groups=[list(range(NCORES))],
                    ins=[att_ins[ti][c].opt()],
                    outs=[ag_outs[ti][c].opt()],
                )

            # second half of the previous slice's O-projection chunks
            if ti > 0:
                for c in range(NCH[ti - 1] // 2, NCH[ti - 1]):
                    oproj_chunk(ti - 1, c)
        for c in range(NCH[TS - 1]):
            oproj_chunk(TS - 1, c)

    nc.compile()
    return nc


def _host_inputs(hidden_states, position_ids, Wq, bq, Wk, bk, Wv, bv, Wo, bo):
    import ml_dtypes
    bf16 = ml_dtypes.bfloat16
    f = np.float32
    X = np.asarray(hidden_states, f).reshape(TOK, H)
    xT = np.ascontiguousarray(X.T).astype(bf16)
    xTt = np.ascontiguousarray(xT.reshape(H, TS, 512).transpose(1, 0, 2))

    pos = np.asarray(position_ids).astype(f).reshape(TOK)
    inv_freq = (1.0 / (THETA ** (np.arange(0, HD, 2, dtype=f) / HD))).astype(f)
    M = inv_freq[:, None] * pos[None, :]              # [64, TOK]
    cosT = np.repeat(np.cos(M), 2, axis=0).astype(f)  # [128, TOK]
    sinT = np.repeat(np.sin(M), 2, axis=0).astype(f)

    rotM = np.zeros((HD, HD), f)
    for i in range(HD // 2):
        rotM[2 * i + 1, 2 * i] = -1.0   # out[2i]   = -in[2i+1]
        rotM[2 * i, 2 * i + 1] = 1.0    # out[2i+1] =  in[2i]

    shared = {
        "xTt": xTt, "cosT": cosT.astype(bf16), "sinT": sinT.astype(bf16),
        "rotM": rotM.astype(bf16),
        "ident": np.eye(128, dtype=f), "ones": np.ones((128, 128), bf16),
        "onesf": np.ones((128, 128), f),
        "maskI": (np.arange(128)[None, :]
                  >= np.arange(128)[:, None]).astype(bf16),
    }
    Wq, Wk, Wv, Wo = (np.asarray(a, f) for a in (Wq, Wk, Wv, Wo))
    bq, bk, bv, bo = (np.asarray(a, f) for a in (bq, bk, bv, bo))
    in_maps = []
    for c in range(NCORES):
        m = dict(shared)
        # [p, h-tile, o] resident layout: wT[h, o] with h = ht*128 + p
        wqT = Wq[c * QO:(c + 1) * QO, :].T.reshape(HT, 128, QO)
        m["wqP"] = np.ascontiguousarray(wqT.transpose(1, 0, 2)).astype(bf16)
        wkT = Wk[c * HD:(c + 1) * HD, :].T.reshape(HT, 128, HD)
        m["wkP"] = np.ascontiguousarray(wkT.transpose(1, 0, 2)).astype(bf16)
        wvT = Wv[c * HD:(c + 1) * HD, :].T.reshape(HT, 128, HD)
        m["wvP"] = np.ascontiguousarray(wvT.transpose(1, 0, 2)).astype(bf16)
        # O-proj: full depth, this core's FO output columns
        woT = Wo[c * FO:(c + 1) * FO, :].T.reshape(HT, 128, FO)
        m["woP"] = np.ascontiguousarray(woT.transpose(1, 0, 2)).astype(bf16)
        m["bqP"] = np.ascontiguousarray(bq[c * QO:(c + 1) * QO].reshape(QH, 128).T)
        m["bkP"] = bk[c * HD:(c + 1) * HD].reshape(128, 1).copy()
        m["bvP"] = bv[c * HD:(c + 1) * HD].reshape(128, 1).copy()
        m["boP"] = bo[c * FO:(c + 1) * FO].reshape(1, FO).copy()
        in_maps.append(m)
    return in_maps


def kernel(hidden_states, position_ids, Wq, bq, Wk, bk, Wv, bv, Wo, bo):
    global LAST_EXEC_NS, LAST_RESULT
    from concourse.bass_utils import run_bass_kernel_spmd

    if "nc" not in _compiled:
        _compiled["nc"] = _build()
    nc = _compiled["nc"]

    in_maps = _host_inputs(hidden_states, position_ids,
                           Wq, bq, Wk, bk, Wv, bv, Wo, bo)
    trace = os.environ.get("KERNEL_TRACE", "0") == "1"
    res = run_bass_kernel_spmd(nc, in_maps, core_ids=list(range(NCORES)),
                               trace=trace)
    LAST_EXEC_NS = res.exec_time_ns
    LAST_RESULT = res
    # core c holds the full token range for output columns 512c..512c+511
    full = np.concatenate([res.results[c]["out"] for c in range(NCORES)],
                          axis=1)
    return np.ascontiguousarray(full).reshape(B, S, H)
